# revision 4
# baseline (speedup 1.0000x reference)
"""v2: sparse expert-parallel MoE on 8 trn2 cores.

Core c computes: its expert's FFN on only the ~2100 tokens routed to it
(host-gathered, padded per token-range to 128-multiples), with routing
weights recomputed on device from the gathered activations; plus a 1/8
tensor-parallel slice of the shared expert over all tokens, gated by an
on-device sigmoid.

Output combine, token-major, one partial [TRANGE, H] per token range:
  1. partials zeroed via DMA (overlaps early compute)
  2. expert pass dma_scatter_add's its weighted rows by in-range token index
     (-1 pads are skipped by the ucode)
  3. shared pass accumulate-DMAs densely, ranges in order
  4. per-range ReduceScatter overlaps trailing shared compute
Host does: routing decision (indices only), gather, reassembly.

The routed-token counts are baked into the compiled program (the problem's
inputs are deterministic); route_host() asserts the capacity still holds.
"""

import numpy as np
from contextlib import ExitStack

import concourse.bass as bass
import concourse.bacc as bacc
import concourse.tile as tile
import concourse.mybir as mybir
from concourse import bass_isa, bass_utils
from concourse.bass_interp import get_hw_module

B, S, H = 2, 4096, 2048
E, TOP_K = 8, 2
I_EXP, I_SH = 1024, 4096
T = B * S
NCORES = 8
I_SLICE = I_SH // NCORES       # 512

P = 128
KT = H // P                    # 16
ITE = I_EXP // P               # 8
ITS = I_SLICE // P             # 4
TB = 512
NBLK = T // TB                 # 16
HH = 512
NHH = H // HH                  # 4
NRANGE = 4
TRANGE = T // NRANGE           # 2048

F32 = mybir.dt.float32
F16 = mybir.dt.float16
I16 = mybir.dt.int16
ALU = mybir.AluOpType
ACTF = mybir.ActivationFunctionType


def build_kernel(pr, group_counts, num_devices=NCORES, with_rs=True,
                 do_expert=True, do_shared=True, debug_out=False):
    """pr: per-range padded counts (multiples of 128, same on all cores).
    group_counts: per-128-group valid-row count (0 => skip scatter)."""
    cap = int(np.ceil(sum(pr) / TB) * TB)          # block-padded capacity
    n_groups = cap // P
    nbe = cap // TB
    # group -> range mapping from pr prefix sums
    g_range = []
    acc = 0
    bounds = np.cumsum(pr)
    for g in range(n_groups):
        lo = g * P
        r = int(np.searchsorted(bounds, lo, side="right"))
        g_range.append(r if r < NRANGE else None)   # None => trailing pad group

    nc = bacc.Bacc(
        "TRN2", target_bir_lowering=False, debug=False, enable_asserts=False,
        num_devices=num_devices, num_swdge_queues=4,
    )
    xT16 = nc.dram_tensor("xT16", [KT, P, T], F16, kind="ExternalInput").ap()
    xTe16 = nc.dram_tensor("xTe16", [KT, P, cap], F16, kind="ExternalInput").ap()
    w9g = nc.dram_tensor("w9g", [KT, P, E], F16, kind="ExternalInput").ap()
    wseg = nc.dram_tensor("wseg", [KT, P, 1], F16, kind="ExternalInput").ap()
    w1e = nc.dram_tensor("w1e", [KT, P, I_EXP], F16, kind="ExternalInput").ap()
    w2e = nc.dram_tensor("w2e", [KT, P, I_EXP], F16, kind="ExternalInput").ap()
    w3e = nc.dram_tensor("w3e", [ITE, P, H], F16, kind="ExternalInput").ap()
    w1s = nc.dram_tensor("w1s", [KT, P, I_SLICE], F16, kind="ExternalInput").ap()
    w2s = nc.dram_tensor("w2s", [KT, P, I_SLICE], F16, kind="ExternalInput").ap()
    w3s = nc.dram_tensor("w3s", [ITS, P, H], F16, kind="ExternalInput").ap()
    sel8d = nc.dram_tensor("sel8", [E, 1], F32, kind="ExternalInput").ap()
    idx16d = nc.dram_tensor("idx16", [P, cap // 16], I16, kind="ExternalInput").ap()
    out_shard = nc.dram_tensor(
        "out_shard", [NRANGE, TRANGE // NCORES, H], F32, kind="ExternalOutput"
    ).ap()

    with tile.TileContext(nc) as tc, ExitStack() as ctx:
        dram = ctx.enter_context(tc.tile_pool(name="dram", bufs=1, space="DRAM"))
        # rows [TRANGE, TRANGE+P) are a garbage region for equalizing pads
        partials = [
            dram.tile([TRANGE + P, H], F32, tag=f"partial{r}", name=f"partial{r}")
            for r in range(NRANGE)
        ]

        # ---- zero the partial buffers -------------------------------------
        with tc.tile_pool(name="sbZ", bufs=1) as sbZ:
            ztile = sbZ.tile([P, H], F32, tag="ztile")
            nc.vector.memset(ztile[:], 0.0)
            for r in range(NRANGE):
                for i in range(TRANGE // P):
                    nc.sync.dma_start(partials[r][i * P : (i + 1) * P, :], ztile[:])

        if do_expert:
            _expert_pass(nc, tc, partials, xTe16, w9g, w1e, w2e, w3e, sel8d,
                         idx16d, cap, nbe, g_range, group_counts)
        if do_shared:
            _shared_pass(nc, tc, partials, xT16, wseg, w1s, w2s, w3s)

        # ---- per-range ReduceScatter + output -----------------------------
        with tc.tile_pool(name="dramR", bufs=1, space="DRAM") as dramR:
            if with_rs:
                for r in range(NRANGE):
                    rs_out = dramR.tile(
                        [TRANGE // NCORES, H], F32, tag=f"rsout{r}", name=f"rsout{r}"
                    )
                    nc.gpsimd.collective_compute(
                        "ReduceScatter",
                        ALU.add,
                        replica_groups=[list(range(NCORES))],
                        ins=[partials[r][0:TRANGE, :].opt()],
                        outs=[rs_out.opt()],
                    )
                    nc.sync.dma_start(out_shard[r], rs_out[:])
            elif debug_out:
                dbg = nc.dram_tensor(
                    "partial_dbg", [T, H], F32, kind="ExternalOutput"
                ).ap()
                for r in range(NRANGE):
                    nc.sync.dma_start(
                        dbg[r * TRANGE : (r + 1) * TRANGE, :],
                        partials[r][0:TRANGE, :],
                    )
            else:
                for r in range(NRANGE):
                    nc.sync.dma_start(
                        out_shard[r], partials[r][0 : TRANGE // NCORES, :]
                    )

    nc.compile()
    return nc


def _expert_pass(nc, tc, partials, xTe16, w9g, w1e, w2e, w3e, sel8d, idx16d,
                 cap, nbe, g_range, group_counts):
    with (
        tc.tile_pool(name="cstE", bufs=1) as cst,
        tc.tile_pool(name="sbXE", bufs=2) as sbX,
        tc.tile_pool(name="sbHE", bufs=1) as sbH,
        tc.tile_pool(name="sbW3E", bufs=2) as sbW3,
        tc.tile_pool(name="sbTE", bufs=3) as sbT,
        tc.tile_pool(name="sbYE", bufs=1) as sbY,
        tc.tile_pool(name="psE", bufs=2, space="PSUM") as ps,
    ):
        w1sb = cst.tile([P, KT, I_EXP], F16, tag="w1sb")
        nc.sync.dma_start(w1sb[:], w1e.rearrange("k p n -> p k n"))
        w2sb = cst.tile([P, KT, I_EXP], F16, tag="w2sb")
        nc.sync.dma_start(w2sb[:], w2e.rearrange("k p n -> p k n"))
        w9sb = cst.tile([P, KT, E], F16, tag="w9sb")
        nc.sync.dma_start(w9sb[:], w9g.rearrange("k p n -> p k n"))
        sel8sb = cst.tile([E, 1], F32, tag="sel8sb")
        nc.sync.dma_start(sel8sb[:], sel8d)
        idxsb = cst.tile([P, cap // 16], I16, tag="idxsb")
        nc.sync.dma_start(idxsb[:], idx16d)

        for eb in range(nbe):
            bsl = slice(eb * TB, (eb + 1) * TB)
            xb = sbX.tile([P, KT, TB], F16, tag="xbe")
            nc.sync.dma_start(xb[:], xTe16[:, :, bsl].rearrange("k p t -> p k t"))

            # recompute this expert's routing weight for gathered tokens:
            # comb = softmax(logits)[e]  (token is in top-2 by construction)
            ps_l = ps.tile([E, TB], F32, tag="ps_l")
            for ko in range(KT):
                nc.tensor.matmul(
                    ps_l[:], w9sb[:, ko, :], xb[:, ko],
                    start=(ko == 0), stop=(ko == KT - 1),
                )
            ex = sbT.tile([E, TB], F32, tag="ex")
            nc.scalar.activation(ex[:], ps_l[:], ACTF.Exp)
            sm = sbT.tile([E, TB], F32, tag="sm")
            nc.gpsimd.partition_all_reduce(
                sm[:], ex[:], channels=E, reduce_op=bass_isa.ReduceOp.add
            )
            rc = sbT.tile([E, TB], F32, tag="rc")
            nc.vector.reciprocal(rc[:], sm[:])
            pr_ = sbT.tile([E, TB], F32, tag="pr")
            nc.vector.tensor_mul(out=pr_[:], in0=ex[:], in1=rc[:])
            nc.vector.tensor_tensor(
                pr_[:], pr_[:], sel8sb[:, 0:1].to_broadcast([E, TB]), ALU.mult
            )
            ce = sbT.tile([E, TB], F32, tag="ce")
            nc.gpsimd.partition_all_reduce(
                ce[:], pr_[:], channels=E, reduce_op=bass_isa.ReduceOp.add
            )
            bc_e = sbT.tile([P, TB], F32, tag="bc_e")
            nc.gpsimd.partition_broadcast(bc_e[:], ce[0:1, :], channels=P)

            hc = sbH.tile([P, ITE, TB], F16, tag="hce")
            for it in range(ITE):
                isl = slice(it * P, (it + 1) * P)
                psG = ps.tile([P, TB], F32, tag="psG")
                psU = ps.tile([P, TB], F32, tag="psU")
                for ko in range(KT):
                    nc.tensor.matmul(
                        psG[:], w1sb[:, ko, isl], xb[:, ko],
                        start=(ko == 0), stop=(ko == KT - 1),
                    )
                for ko in range(KT):
                    nc.tensor.matmul(
                        psU[:], w2sb[:, ko, isl], xb[:, ko],
                        start=(ko == 0), stop=(ko == KT - 1),
                    )
                sg = sbT.tile([P, TB], F32, tag="sg")
                nc.scalar.activation(sg[:], psG[:], ACTF.Silu)
                nc.vector.tensor_mul(out=sg[:], in0=sg[:], in1=psU[:])
                nc.vector.tensor_tensor(hc[:, it], sg[:], bc_e[:], ALU.mult)

            # token-major down-proj: yf[tok, h]
            yf = sbY.tile([P, TB // P, NHH * HH], F32, tag="yf")
            for hh in range(NHH):
                w3t = sbW3.tile([P, ITE, HH], F16, tag="w3te")
                nc.sync.dma_start(
                    w3t[:],
                    w3e[:, :, hh * HH : (hh + 1) * HH].rearrange("i p h -> p i h"),
                )
                for ts_ in range(TB // P):
                    psY = ps.tile([P, HH], F32, tag="psY")
                    for it in range(ITE):
                        nc.tensor.matmul(
                            psY[:],
                            hc[:, it, ts_ * P : (ts_ + 1) * P],
                            w3t[:, it, :],
                            start=(it == 0), stop=(it == ITE - 1),
                        )
                    nc.vector.tensor_copy(
                        yf[:, ts_, hh * HH : (hh + 1) * HH], psY[:]
                    )
            # scatter-add 128-row groups into their range's partial
            for ts_ in range(TB // P):
                g = eb * (TB // P) + ts_
                r = g_range[g]
                nvalid = group_counts[g]
                if r is None or nvalid == 0:
                    continue
                nc.gpsimd.dma_scatter_add(
                    out_ap=partials[r][:, :],
                    in_ap=yf[:, ts_ : ts_ + 1, :],
                    idxs_ap=idxsb[:, g * 8 : (g + 1) * 8],
                    num_idxs=P,
                    num_idxs_reg=int(nvalid),
                    elem_size=H,
                    queue_num=g % 4,
                )


def _shared_pass(nc, tc, partials, xT16, wseg, w1s, w2s, w3s):
    with (
        tc.tile_pool(name="cstS", bufs=1) as cst,
        tc.tile_pool(name="sbXS", bufs=2) as sbX,
        tc.tile_pool(name="sbHS", bufs=2) as sbH,
        tc.tile_pool(name="sbW3S", bufs=2) as sbW3,
        tc.tile_pool(name="sbTS", bufs=3) as sbT,
        tc.tile_pool(name="sbYS", bufs=3) as sbY,
        tc.tile_pool(name="psS", bufs=2, space="PSUM") as ps,
    ):
        w1sb = cst.tile([P, KT, I_SLICE], F16, tag="w1sbs")
        nc.sync.dma_start(w1sb[:], w1s.rearrange("k p n -> p k n"))
        w2sb = cst.tile([P, KT, I_SLICE], F16, tag="w2sbs")
        nc.sync.dma_start(w2sb[:], w2s.rearrange("k p n -> p k n"))
        wgsb = cst.tile([P, KT, 1], F16, tag="wgsb")
        nc.sync.dma_start(wgsb[:], wseg.rearrange("k p n -> p k n"))

        for b in range(NBLK):
            bsl = slice(b * TB, (b + 1) * TB)
            rng_i = (b * TB) // TRANGE
            xb = sbX.tile([P, KT, TB], F16, tag="xbs")
            nc.sync.dma_start(xb[:], xT16[:, :, bsl].rearrange("k p t -> p k t"))

            ps_g = ps.tile([1, TB], F32, tag="ps_g")
            for ko in range(KT):
                nc.tensor.matmul(
                    ps_g[:], wgsb[:, ko, :], xb[:, ko],
                    start=(ko == 0), stop=(ko == KT - 1),
                )
            srow = sbT.tile([1, TB], F32, tag="srow")
            nc.scalar.activation(srow[:], ps_g[:], ACTF.Sigmoid)
            bc_s = sbT.tile([P, TB], F32, tag="bc_s")
            nc.gpsimd.partition_broadcast(bc_s[:], srow[:], channels=P)

            hc = sbH.tile([P, ITS, TB], F16, tag="hcs")
            for it in range(ITS):
                isl = slice(it * P, (it + 1) * P)
                psG = ps.tile([P, TB], F32, tag="psGs")
                psU = ps.tile([P, TB], F32, tag="psUs")
                for ko in range(KT):
                    nc.tensor.matmul(
                        psG[:], w1sb[:, ko, isl], xb[:, ko],
                        start=(ko == 0), stop=(ko == KT - 1),
                    )
                for ko in range(KT):
                    nc.tensor.matmul(
                        psU[:], w2sb[:, ko, isl], xb[:, ko],
                        start=(ko == 0), stop=(ko == KT - 1),
                    )
                sg = sbT.tile([P, TB], F32, tag="sgs")
                nc.scalar.activation(sg[:], psG[:], ACTF.Silu)
                nc.vector.tensor_mul(out=sg[:], in0=sg[:], in1=psU[:])
                nc.vector.tensor_tensor(hc[:, it], sg[:], bc_s[:], ALU.mult)

            for hh in range(NHH):
                w3t = sbW3.tile([P, ITS, HH], F16, tag="w3ts")
                nc.sync.dma_start(
                    w3t[:],
                    w3s[:, :, hh * HH : (hh + 1) * HH].rearrange("i p h -> p i h"),
                )
                for ts_ in range(TB // P):
                    psY = ps.tile([P, HH], F32, tag="psYs")
                    for it in range(ITS):
                        nc.tensor.matmul(
                            psY[:],
                            hc[:, it, ts_ * P : (ts_ + 1) * P],
                            w3t[:, it, :],
                            start=(it == 0), stop=(it == ITS - 1),
                        )
                    ys = sbY.tile([P, HH], F32, tag="ys")
                    nc.vector.tensor_copy(ys[:], psY[:])
                    row0 = (b * TB + ts_ * P) % TRANGE
                    nc.gpsimd.dma_start(
                        partials[rng_i][row0 : row0 + P, hh * HH : (hh + 1) * HH],
                        ys[:],
                        accum_op=ALU.add,
                    )


def route_host(inputs):
    """Routing decision only (indices); all values are recomputed on device."""
    x = np.asarray(inputs["hidden_states"], np.float64).reshape(T, H)
    gw = np.asarray(inputs["gate_w"], np.float64)
    logits = x @ gw.T
    p = np.exp(logits - logits.max(-1, keepdims=True))
    p /= p.sum(-1, keepdims=True)
    order = np.argsort(-p, axis=-1, kind="stable")
    top2 = order[:, :TOP_K]
    per_core = []
    for e in range(NCORES):
        toks = np.where((top2 == e).any(-1))[0]
        per_range = [
            toks[(toks >= r * TRANGE) & (toks < (r + 1) * TRANGE)] - r * TRANGE
            for r in range(NRANGE)
        ]
        per_core.append(per_range)
    # per-range padded counts: max over cores, rounded up to 128
    pr = [
        int(np.ceil(max(len(per_core[c][r]) for c in range(NCORES)) / P) * P)
        for r in range(NRANGE)
    ]
    return per_core, pr


def _derive_params(inputs):
    """Per-group scatter counts must be identical across cores (one SPMD
    program).  Each group's count = max over cores; cores with fewer tokens
    pad the window with index TRANGE (the garbage row region), so every core
    has exactly `count` non-negative entries followed by -1s."""
    per_core, pr = route_host(inputs)
    cap = int(np.ceil(sum(pr) / TB) * TB)
    n_groups = cap // P
    bounds = np.cumsum(pr)
    group_counts = []
    for g in range(n_groups):
        lo = g * P
        r = int(np.searchsorted(bounds, lo, side="right"))
        if r >= NRANGE:
            group_counts.append(0)
            continue
        seg_lo = lo - (int(bounds[r - 1]) if r > 0 else 0)
        mx = max(
            min(max(len(per_core[c][r]) - seg_lo, 0), P) for c in range(NCORES)
        )
        group_counts.append(int(mx))
    return per_core, pr, cap, group_counts


def build_idx_and_gather(per_range_tok, pr, cap, group_counts):
    """Per-core: padded index list (range-relative; surplus entries within a
    group's count point at the garbage row TRANGE; -1 beyond) + absolute
    gather columns (pads gather token 0)."""
    glist = np.full((cap,), -1, np.int16)
    gcols = np.zeros((cap,), np.int64)
    bounds = np.cumsum(pr)
    off = 0
    for r in range(NRANGE):
        toks = per_range_tok[r]
        glist[off : off + len(toks)] = toks.astype(np.int16)
        gcols[off : off + len(toks)] = toks + r * TRANGE
        off += pr[r]
    # within each group, raise entries [own_count, group_count) to garbage row
    for g in range(cap // P):
        cnt = group_counts[g]
        w = glist[g * P : (g + 1) * P]
        pad = (w < 0) & (np.arange(P) < cnt)
        w[pad] = TRANGE
    idx16 = np.empty((P, cap // 16), np.int16)
    wrapped = glist.reshape(cap // 16, 16).T        # [16, cap/16]
    for k in range(8):
        idx16[k * 16 : (k + 1) * 16] = wrapped
    return glist, gcols, idx16


def make_in_maps(inputs):
    hs = np.ascontiguousarray(inputs["hidden_states"], dtype=np.float32)
    x = hs.reshape(T, H)
    xT16_flat = np.ascontiguousarray(x.T).astype(np.float16)
    xT16 = xT16_flat.reshape(KT, P, T)

    gate_w = np.asarray(inputs["gate_w"], np.float32)
    seg_w = np.asarray(inputs["shared_expert_gate_w"], np.float32)
    w9g = np.ascontiguousarray(gate_w.T).astype(np.float16).reshape(KT, P, E)
    wseg = np.ascontiguousarray(seg_w.T).astype(np.float16).reshape(KT, P, 1)

    egw = np.asarray(inputs["expert_gate_w"], np.float32)
    euw = np.asarray(inputs["expert_up_w"], np.float32)
    edw = np.asarray(inputs["expert_down_w"], np.float32)
    sgw = np.asarray(inputs["shared_gate_w"], np.float32)
    suw = np.asarray(inputs["shared_up_w"], np.float32)
    sdw = np.asarray(inputs["shared_down_w"], np.float32)

    per_core, pr, cap, group_counts = _derive_params(inputs)

    in_maps = []
    for c in range(NCORES):
        ssl = slice(c * I_SLICE, (c + 1) * I_SLICE)
        glist, gcols, idx16 = build_idx_and_gather(
            per_core[c], pr, cap, group_counts
        )
        xTe16 = np.ascontiguousarray(xT16_flat[:, gcols]).reshape(KT, P, cap)
        sel8 = np.zeros((E, 1), np.float32)
        sel8[c, 0] = 1.0
        in_maps.append(
            {
                "xT16": xT16,
                "xTe16": xTe16,
                "w9g": w9g,
                "wseg": wseg,
                "w1e": np.ascontiguousarray(egw[c].T).astype(np.float16).reshape(KT, P, I_EXP),
                "w2e": np.ascontiguousarray(euw[c].T).astype(np.float16).reshape(KT, P, I_EXP),
                "w3e": np.ascontiguousarray(edw[c].T).astype(np.float16).reshape(ITE, P, H),
                "w1s": np.ascontiguousarray(sgw[ssl].T).astype(np.float16).reshape(KT, P, I_SLICE),
                "w2s": np.ascontiguousarray(suw[ssl].T).astype(np.float16).reshape(KT, P, I_SLICE),
                "w3s": np.ascontiguousarray(sdw[:, ssl].T).astype(np.float16).reshape(ITS, P, H),
                "sel8": sel8,
                "idx16": idx16,
            }
        )
    return in_maps, pr, group_counts


def assemble_output(results):
    out = np.empty((T, H), np.float32)
    rows = TRANGE // NCORES
    for c in range(NCORES):
        sh = results[c]["out_shard"]
        for r in range(NRANGE):
            base = r * TRANGE + c * rows
            out[base : base + rows] = sh[r]
    return out.reshape(B, S, H)


_nc_cache = {}


def kernel(**inputs) -> np.ndarray:
    in_maps, pr, group_counts = make_in_maps(inputs)
    key = (tuple(pr), tuple(group_counts))
    if key not in _nc_cache:
        nc = build_kernel(pr, group_counts)
        nc.m = get_hw_module(nc.m)
        _nc_cache[key] = nc
    nc = _nc_cache[key]
    res = bass_utils.run_bass_kernel_spmd(
        nc, in_maps, core_ids=list(range(NCORES))
    )
    return assemble_output(res.results)


# revision 5
# speedup vs baseline: 1.0746x; 1.0746x over previous
"""v2: sparse expert-parallel MoE on 8 trn2 cores.

Core c computes: its expert's FFN on only the ~2100 tokens routed to it
(host-gathered, padded per token-range to 128-multiples), with routing
weights recomputed on device from the gathered activations; plus a 1/8
tensor-parallel slice of the shared expert over all tokens, gated by an
on-device sigmoid.

Output combine, token-major, one partial [TRANGE, H] per token range:
  1. partials zeroed via DMA (overlaps early compute)
  2. expert pass dma_scatter_add's its weighted rows by in-range token index
     (-1 pads are skipped by the ucode)
  3. shared pass accumulate-DMAs densely, ranges in order
  4. per-range ReduceScatter overlaps trailing shared compute
Host does: routing decision (indices only), gather, reassembly.

The routed-token counts are baked into the compiled program (the problem's
inputs are deterministic); route_host() asserts the capacity still holds.
"""

import numpy as np
from contextlib import ExitStack

import concourse.bass as bass
import concourse.bacc as bacc
import concourse.tile as tile
import concourse.mybir as mybir
from concourse import bass_isa, bass_utils
from concourse.bass_interp import get_hw_module

B, S, H = 2, 4096, 2048
E, TOP_K = 8, 2
I_EXP, I_SH = 1024, 4096
T = B * S
NCORES = 8
I_SLICE = I_SH // NCORES       # 512

P = 128
KT = H // P                    # 16
ITE = I_EXP // P               # 8
ITS = I_SLICE // P             # 4
TB = 512
NBLK = T // TB                 # 16
HH = 512
NHH = H // HH                  # 4
NRANGE = 4
TRANGE = T // NRANGE           # 2048

F32 = mybir.dt.float32
F16 = mybir.dt.float16
I16 = mybir.dt.int16
ALU = mybir.AluOpType
ACTF = mybir.ActivationFunctionType


def build_kernel(pr, group_counts, num_devices=NCORES, with_rs=True,
                 do_expert=True, do_shared=True, debug_out=False):
    """pr: per-range padded counts (multiples of 128, same on all cores).
    group_counts: per-128-group valid-row count (0 => skip scatter)."""
    cap = int(np.ceil(sum(pr) / TB) * TB)          # block-padded capacity
    n_groups = cap // P
    nbe = cap // TB
    # group -> range mapping from pr prefix sums
    g_range = []
    acc = 0
    bounds = np.cumsum(pr)
    for g in range(n_groups):
        lo = g * P
        r = int(np.searchsorted(bounds, lo, side="right"))
        g_range.append(r if r < NRANGE else None)   # None => trailing pad group

    nc = bacc.Bacc(
        "TRN2", target_bir_lowering=False, debug=False, enable_asserts=False,
        num_devices=num_devices, num_swdge_queues=4,
    )
    xT16 = nc.dram_tensor("xT16", [KT, P, T], F16, kind="ExternalInput").ap()
    xTe16 = nc.dram_tensor("xTe16", [KT, P, cap], F16, kind="ExternalInput").ap()
    w9g = nc.dram_tensor("w9g", [KT, P, E], F16, kind="ExternalInput").ap()
    wseg = nc.dram_tensor("wseg", [KT, P, 1], F16, kind="ExternalInput").ap()
    w1e = nc.dram_tensor("w1e", [KT, P, I_EXP], F16, kind="ExternalInput").ap()
    w2e = nc.dram_tensor("w2e", [KT, P, I_EXP], F16, kind="ExternalInput").ap()
    w3e = nc.dram_tensor("w3e", [ITE, P, H], F16, kind="ExternalInput").ap()
    w1s = nc.dram_tensor("w1s", [KT, P, I_SLICE], F16, kind="ExternalInput").ap()
    w2s = nc.dram_tensor("w2s", [KT, P, I_SLICE], F16, kind="ExternalInput").ap()
    w3s = nc.dram_tensor("w3s", [ITS, P, H], F16, kind="ExternalInput").ap()
    sel8d = nc.dram_tensor("sel8", [E, 1], F32, kind="ExternalInput").ap()
    idx16d = nc.dram_tensor("idx16", [P, cap // 16], I16, kind="ExternalInput").ap()
    out_shard = nc.dram_tensor(
        "out_shard", [NRANGE, TRANGE // NCORES, H], F32, kind="ExternalOutput"
    ).ap()

    with tile.TileContext(nc) as tc, ExitStack() as ctx:
        dram = ctx.enter_context(tc.tile_pool(name="dram", bufs=1, space="DRAM"))
        # rows [TRANGE, TRANGE+P) are a garbage region for equalizing pads
        partials = [
            dram.tile([TRANGE + P, H], F32, tag=f"partial{r}", name=f"partial{r}")
            for r in range(NRANGE)
        ]

        # ---- zero the partial buffers -------------------------------------
        with tc.tile_pool(name="sbZ", bufs=1) as sbZ:
            ztile = sbZ.tile([P, H], F32, tag="ztile")
            nc.vector.memset(ztile[:], 0.0)
            for r in range(NRANGE):
                for i in range(TRANGE // P):
                    nc.sync.dma_start(partials[r][i * P : (i + 1) * P, :], ztile[:])

        if do_expert:
            _expert_pass(nc, tc, partials, xTe16, w9g, w1e, w2e, w3e, sel8d,
                         idx16d, cap, nbe, g_range, group_counts)
        if do_shared:
            _shared_pass(nc, tc, partials, xT16, wseg, w1s, w2s, w3s)

        # ---- per-range ReduceScatter + output -----------------------------
        with tc.tile_pool(name="dramR", bufs=1, space="DRAM") as dramR:
            if with_rs:
                for r in range(NRANGE):
                    rs_out = dramR.tile(
                        [TRANGE // NCORES, H], F32, tag=f"rsout{r}", name=f"rsout{r}"
                    )
                    nc.gpsimd.collective_compute(
                        "ReduceScatter",
                        ALU.add,
                        replica_groups=[list(range(NCORES))],
                        ins=[partials[r][0:TRANGE, :].opt()],
                        outs=[rs_out.opt()],
                    )
                    nc.sync.dma_start(out_shard[r], rs_out[:])
            elif debug_out:
                dbg = nc.dram_tensor(
                    "partial_dbg", [T, H], F32, kind="ExternalOutput"
                ).ap()
                for r in range(NRANGE):
                    nc.sync.dma_start(
                        dbg[r * TRANGE : (r + 1) * TRANGE, :],
                        partials[r][0:TRANGE, :],
                    )
            else:
                for r in range(NRANGE):
                    nc.sync.dma_start(
                        out_shard[r], partials[r][0 : TRANGE // NCORES, :]
                    )

    nc.compile()
    return nc


def _expert_pass(nc, tc, partials, xTe16, w9g, w1e, w2e, w3e, sel8d, idx16d,
                 cap, nbe, g_range, group_counts):
    with (
        tc.tile_pool(name="cstE", bufs=1) as cst,
        tc.tile_pool(name="sbXE", bufs=2) as sbX,
        tc.tile_pool(name="sbHE", bufs=1) as sbH,
        tc.tile_pool(name="sbW3E", bufs=2) as sbW3,
        tc.tile_pool(name="sbTE", bufs=3) as sbT,
        tc.tile_pool(name="sbYE", bufs=1) as sbY,
        tc.tile_pool(name="psE", bufs=2, space="PSUM") as ps,
    ):
        w1sb = cst.tile([P, KT, I_EXP], F16, tag="w1sb")
        nc.sync.dma_start(w1sb[:], w1e.rearrange("k p n -> p k n"))
        w2sb = cst.tile([P, KT, I_EXP], F16, tag="w2sb")
        nc.sync.dma_start(w2sb[:], w2e.rearrange("k p n -> p k n"))
        w9sb = cst.tile([P, KT, E], F16, tag="w9sb")
        nc.sync.dma_start(w9sb[:], w9g.rearrange("k p n -> p k n"))
        sel8sb = cst.tile([E, 1], F32, tag="sel8sb")
        nc.sync.dma_start(sel8sb[:], sel8d)
        idxsb = cst.tile([P, cap // 16], I16, tag="idxsb")
        nc.sync.dma_start(idxsb[:], idx16d)

        for eb in range(nbe):
            bsl = slice(eb * TB, (eb + 1) * TB)
            xb = sbX.tile([P, KT, TB], F16, tag="xbe")
            nc.sync.dma_start(xb[:], xTe16[:, :, bsl].rearrange("k p t -> p k t"))

            # recompute this expert's routing weight for gathered tokens:
            # comb = softmax(logits)[e]  (token is in top-2 by construction)
            ps_l = ps.tile([E, TB], F32, tag="ps_l")
            for ko in range(KT):
                nc.tensor.matmul(
                    ps_l[:], w9sb[:, ko, :], xb[:, ko],
                    start=(ko == 0), stop=(ko == KT - 1),
                )
            ex = sbT.tile([E, TB], F32, tag="ex")
            nc.scalar.activation(ex[:], ps_l[:], ACTF.Exp)
            sm = sbT.tile([E, TB], F32, tag="sm")
            nc.gpsimd.partition_all_reduce(
                sm[:], ex[:], channels=E, reduce_op=bass_isa.ReduceOp.add
            )
            rc = sbT.tile([E, TB], F32, tag="rc")
            nc.vector.reciprocal(rc[:], sm[:])
            pr_ = sbT.tile([E, TB], F32, tag="pr")
            nc.vector.tensor_mul(out=pr_[:], in0=ex[:], in1=rc[:])
            nc.vector.tensor_tensor(
                pr_[:], pr_[:], sel8sb[:, 0:1].to_broadcast([E, TB]), ALU.mult
            )
            ce = sbT.tile([E, TB], F32, tag="ce")
            nc.gpsimd.partition_all_reduce(
                ce[:], pr_[:], channels=E, reduce_op=bass_isa.ReduceOp.add
            )
            bc_e = sbT.tile([P, TB], F32, tag="bc_e")
            nc.gpsimd.partition_broadcast(bc_e[:], ce[0:1, :], channels=P)

            hc = sbH.tile([P, ITE, TB], F16, tag="hce")
            for it in range(ITE):
                isl = slice(it * P, (it + 1) * P)
                psG = ps.tile([P, TB], F32, tag="psG")
                psU = ps.tile([P, TB], F32, tag="psU")
                for ko in range(KT):
                    nc.tensor.matmul(
                        psG[:], w1sb[:, ko, isl], xb[:, ko],
                        start=(ko == 0), stop=(ko == KT - 1),
                    )
                for ko in range(KT):
                    nc.tensor.matmul(
                        psU[:], w2sb[:, ko, isl], xb[:, ko],
                        start=(ko == 0), stop=(ko == KT - 1),
                    )
                sg = sbT.tile([P, TB], F32, tag="sg")
                nc.scalar.activation(sg[:], psG[:], ACTF.Silu)
                nc.vector.tensor_mul(out=sg[:], in0=sg[:], in1=psU[:])
                nc.vector.tensor_tensor(hc[:, it], sg[:], bc_e[:], ALU.mult)

            # token-major down-proj: yf[tok, h]
            yf = sbY.tile([P, TB // P, NHH * HH], F32, tag="yf")
            for hh in range(NHH):
                w3t = sbW3.tile([P, ITE, HH], F16, tag="w3te")
                nc.sync.dma_start(
                    w3t[:],
                    w3e[:, :, hh * HH : (hh + 1) * HH].rearrange("i p h -> p i h"),
                )
                for ts_ in range(TB // P):
                    psY = ps.tile([P, HH], F32, tag="psY")
                    for it in range(ITE):
                        nc.tensor.matmul(
                            psY[:],
                            hc[:, it, ts_ * P : (ts_ + 1) * P],
                            w3t[:, it, :],
                            start=(it == 0), stop=(it == ITE - 1),
                        )
                    nc.vector.tensor_copy(
                        yf[:, ts_, hh * HH : (hh + 1) * HH], psY[:]
                    )
            # scatter-add 128-row groups into their range's partial
            for ts_ in range(TB // P):
                g = eb * (TB // P) + ts_
                r = g_range[g]
                nvalid = group_counts[g]
                if r is None or nvalid == 0:
                    continue
                nc.gpsimd.dma_scatter_add(
                    out_ap=partials[r][:, :],
                    in_ap=yf[:, ts_ : ts_ + 1, :],
                    idxs_ap=idxsb[:, g * 8 : (g + 1) * 8],
                    num_idxs=P,
                    num_idxs_reg=int(nvalid),
                    elem_size=H,
                    queue_num=g % 4,
                )


def _shared_pass(nc, tc, partials, xT16, wseg, w1s, w2s, w3s):
    with (
        tc.tile_pool(name="cstS", bufs=1) as cst,
        tc.tile_pool(name="sbXS", bufs=2) as sbX,
        tc.tile_pool(name="sbHS", bufs=2) as sbH,
        tc.tile_pool(name="sbW3S", bufs=2) as sbW3,
        tc.tile_pool(name="sbTS", bufs=3) as sbT,
        tc.tile_pool(name="sbYS", bufs=3) as sbY,
        tc.tile_pool(name="psS", bufs=2, space="PSUM") as ps,
    ):
        w1sb = cst.tile([P, KT, I_SLICE], F16, tag="w1sbs")
        nc.sync.dma_start(w1sb[:], w1s.rearrange("k p n -> p k n"))
        w2sb = cst.tile([P, KT, I_SLICE], F16, tag="w2sbs")
        nc.sync.dma_start(w2sb[:], w2s.rearrange("k p n -> p k n"))
        wgsb = cst.tile([P, KT, 1], F16, tag="wgsb")
        nc.sync.dma_start(wgsb[:], wseg.rearrange("k p n -> p k n"))

        for b in range(NBLK):
            bsl = slice(b * TB, (b + 1) * TB)
            rng_i = (b * TB) // TRANGE
            xb = sbX.tile([P, KT, TB], F16, tag="xbs")
            nc.sync.dma_start(xb[:], xT16[:, :, bsl].rearrange("k p t -> p k t"))

            ps_g = ps.tile([1, TB], F32, tag="ps_g")
            for ko in range(KT):
                nc.tensor.matmul(
                    ps_g[:], wgsb[:, ko, :], xb[:, ko],
                    start=(ko == 0), stop=(ko == KT - 1),
                )
            srow = sbT.tile([1, TB], F32, tag="srow")
            nc.scalar.activation(srow[:], ps_g[:], ACTF.Sigmoid)
            bc_s = sbT.tile([P, TB], F32, tag="bc_s")
            nc.gpsimd.partition_broadcast(bc_s[:], srow[:], channels=P)

            hc = sbH.tile([P, ITS, TB], F16, tag="hcs")
            for it in range(ITS):
                isl = slice(it * P, (it + 1) * P)
                psG = ps.tile([P, TB], F32, tag="psGs")
                psU = ps.tile([P, TB], F32, tag="psUs")
                for ko in range(KT):
                    nc.tensor.matmul(
                        psG[:], w1sb[:, ko, isl], xb[:, ko],
                        start=(ko == 0), stop=(ko == KT - 1),
                    )
                for ko in range(KT):
                    nc.tensor.matmul(
                        psU[:], w2sb[:, ko, isl], xb[:, ko],
                        start=(ko == 0), stop=(ko == KT - 1),
                    )
                sg = sbT.tile([P, TB], F32, tag="sgs")
                nc.scalar.activation(sg[:], psG[:], ACTF.Silu)
                nc.vector.tensor_mul(out=sg[:], in0=sg[:], in1=psU[:])
                nc.vector.tensor_tensor(hc[:, it], sg[:], bc_s[:], ALU.mult)

            # stage the whole block's [TB, H] in SBUF, then one accum-DMA
            # (per-tile accum-DMAs each hold the GPSIMD engine ~1us for
            # SWDGE descriptor generation -- 256 of them serialize)
            yb = sbY.tile([P, TB // P, H], F32, tag="yblk")
            for hh in range(NHH):
                w3t = sbW3.tile([P, ITS, HH], F16, tag="w3ts")
                nc.sync.dma_start(
                    w3t[:],
                    w3s[:, :, hh * HH : (hh + 1) * HH].rearrange("i p h -> p i h"),
                )
                for ts_ in range(TB // P):
                    psY = ps.tile([P, HH], F32, tag="psYs")
                    for it in range(ITS):
                        nc.tensor.matmul(
                            psY[:],
                            hc[:, it, ts_ * P : (ts_ + 1) * P],
                            w3t[:, it, :],
                            start=(it == 0), stop=(it == ITS - 1),
                        )
                    nc.vector.tensor_copy(
                        yb[:, ts_, hh * HH : (hh + 1) * HH], psY[:]
                    )
            row0 = (b * TB) % TRANGE
            nc.gpsimd.dma_start(
                partials[rng_i][row0 : row0 + TB, :].rearrange(
                    "(a p) h -> p a h", p=P
                ),
                yb[:],
                accum_op=ALU.add,
            )


def route_host(inputs):
    """Routing decision only (indices); all values are recomputed on device."""
    x = np.asarray(inputs["hidden_states"], np.float64).reshape(T, H)
    gw = np.asarray(inputs["gate_w"], np.float64)
    logits = x @ gw.T
    p = np.exp(logits - logits.max(-1, keepdims=True))
    p /= p.sum(-1, keepdims=True)
    order = np.argsort(-p, axis=-1, kind="stable")
    top2 = order[:, :TOP_K]
    per_core = []
    for e in range(NCORES):
        toks = np.where((top2 == e).any(-1))[0]
        per_range = [
            toks[(toks >= r * TRANGE) & (toks < (r + 1) * TRANGE)] - r * TRANGE
            for r in range(NRANGE)
        ]
        per_core.append(per_range)
    # per-range padded counts: max over cores, rounded up to 128
    pr = [
        int(np.ceil(max(len(per_core[c][r]) for c in range(NCORES)) / P) * P)
        for r in range(NRANGE)
    ]
    return per_core, pr


def _derive_params(inputs):
    """Per-group scatter counts must be identical across cores (one SPMD
    program).  Each group's count = max over cores; cores with fewer tokens
    pad the window with index TRANGE (the garbage row region), so every core
    has exactly `count` non-negative entries followed by -1s."""
    per_core, pr = route_host(inputs)
    cap = int(np.ceil(sum(pr) / TB) * TB)
    n_groups = cap // P
    bounds = np.cumsum(pr)
    group_counts = []
    for g in range(n_groups):
        lo = g * P
        r = int(np.searchsorted(bounds, lo, side="right"))
        if r >= NRANGE:
            group_counts.append(0)
            continue
        seg_lo = lo - (int(bounds[r - 1]) if r > 0 else 0)
        mx = max(
            min(max(len(per_core[c][r]) - seg_lo, 0), P) for c in range(NCORES)
        )
        group_counts.append(int(mx))
    return per_core, pr, cap, group_counts


def build_idx_and_gather(per_range_tok, pr, cap, group_counts):
    """Per-core: padded index list (range-relative; surplus entries within a
    group's count point at the garbage row TRANGE; -1 beyond) + absolute
    gather columns (pads gather token 0)."""
    glist = np.full((cap,), -1, np.int16)
    gcols = np.zeros((cap,), np.int64)
    bounds = np.cumsum(pr)
    off = 0
    for r in range(NRANGE):
        toks = per_range_tok[r]
        glist[off : off + len(toks)] = toks.astype(np.int16)
        gcols[off : off + len(toks)] = toks + r * TRANGE
        off += pr[r]
    # within each group, raise entries [own_count, group_count) to garbage row
    for g in range(cap // P):
        cnt = group_counts[g]
        w = glist[g * P : (g + 1) * P]
        pad = (w < 0) & (np.arange(P) < cnt)
        w[pad] = TRANGE
    idx16 = np.empty((P, cap // 16), np.int16)
    wrapped = glist.reshape(cap // 16, 16).T        # [16, cap/16]
    for k in range(8):
        idx16[k * 16 : (k + 1) * 16] = wrapped
    return glist, gcols, idx16


def make_in_maps(inputs):
    hs = np.ascontiguousarray(inputs["hidden_states"], dtype=np.float32)
    x = hs.reshape(T, H)
    xT16_flat = np.ascontiguousarray(x.T).astype(np.float16)
    xT16 = xT16_flat.reshape(KT, P, T)

    gate_w = np.asarray(inputs["gate_w"], np.float32)
    seg_w = np.asarray(inputs["shared_expert_gate_w"], np.float32)
    w9g = np.ascontiguousarray(gate_w.T).astype(np.float16).reshape(KT, P, E)
    wseg = np.ascontiguousarray(seg_w.T).astype(np.float16).reshape(KT, P, 1)

    egw = np.asarray(inputs["expert_gate_w"], np.float32)
    euw = np.asarray(inputs["expert_up_w"], np.float32)
    edw = np.asarray(inputs["expert_down_w"], np.float32)
    sgw = np.asarray(inputs["shared_gate_w"], np.float32)
    suw = np.asarray(inputs["shared_up_w"], np.float32)
    sdw = np.asarray(inputs["shared_down_w"], np.float32)

    per_core, pr, cap, group_counts = _derive_params(inputs)

    in_maps = []
    for c in range(NCORES):
        ssl = slice(c * I_SLICE, (c + 1) * I_SLICE)
        glist, gcols, idx16 = build_idx_and_gather(
            per_core[c], pr, cap, group_counts
        )
        xTe16 = np.ascontiguousarray(xT16_flat[:, gcols]).reshape(KT, P, cap)
        sel8 = np.zeros((E, 1), np.float32)
        sel8[c, 0] = 1.0
        in_maps.append(
            {
                "xT16": xT16,
                "xTe16": xTe16,
                "w9g": w9g,
                "wseg": wseg,
                "w1e": np.ascontiguousarray(egw[c].T).astype(np.float16).reshape(KT, P, I_EXP),
                "w2e": np.ascontiguousarray(euw[c].T).astype(np.float16).reshape(KT, P, I_EXP),
                "w3e": np.ascontiguousarray(edw[c].T).astype(np.float16).reshape(ITE, P, H),
                "w1s": np.ascontiguousarray(sgw[ssl].T).astype(np.float16).reshape(KT, P, I_SLICE),
                "w2s": np.ascontiguousarray(suw[ssl].T).astype(np.float16).reshape(KT, P, I_SLICE),
                "w3s": np.ascontiguousarray(sdw[:, ssl].T).astype(np.float16).reshape(ITS, P, H),
                "sel8": sel8,
                "idx16": idx16,
            }
        )
    return in_maps, pr, group_counts


def assemble_output(results):
    out = np.empty((T, H), np.float32)
    rows = TRANGE // NCORES
    for c in range(NCORES):
        sh = results[c]["out_shard"]
        for r in range(NRANGE):
            base = r * TRANGE + c * rows
            out[base : base + rows] = sh[r]
    return out.reshape(B, S, H)


_nc_cache = {}


def kernel(**inputs) -> np.ndarray:
    in_maps, pr, group_counts = make_in_maps(inputs)
    key = (tuple(pr), tuple(group_counts))
    if key not in _nc_cache:
        nc = build_kernel(pr, group_counts)
        nc.m = get_hw_module(nc.m)
        _nc_cache[key] = nc
    nc = _nc_cache[key]
    res = bass_utils.run_bass_kernel_spmd(
        nc, in_maps, core_ids=list(range(NCORES))
    )
    return assemble_output(res.results)


# revision 13
# speedup vs baseline: 1.1091x; 1.0320x over previous
"""v2: sparse expert-parallel MoE on 8 trn2 cores.

Core c computes: its expert's FFN on only the ~2100 tokens routed to it
(host-gathered, padded per token-range to 128-multiples), with routing
weights recomputed on device from the gathered activations; plus a 1/8
tensor-parallel slice of the shared expert over all tokens, gated by an
on-device sigmoid.

Output combine, token-major, one partial [TRANGE, H] per token range:
  1. partials zeroed via DMA (overlaps early compute)
  2. expert pass dma_scatter_add's its weighted rows by in-range token index
     (-1 pads are skipped by the ucode)
  3. shared pass accumulate-DMAs densely, ranges in order
  4. per-range ReduceScatter overlaps trailing shared compute
Host does: routing decision (indices only), gather, reassembly.

The per-range routed-token counts are derived from the actual inputs and
baked into the compiled program (compilation happens inside kernel(), after
routing), so capacity always fits; the build is cached keyed on the counts.
"""

import numpy as np
from contextlib import ExitStack

import concourse.bass as bass
import concourse.bacc as bacc
import concourse.tile as tile
import concourse.mybir as mybir
from concourse import bass_isa, bass_utils
from concourse.bass_interp import get_hw_module

B, S, H = 2, 4096, 2048
E, TOP_K = 8, 2
I_EXP, I_SH = 1024, 4096
T = B * S
NCORES = 8
I_SLICE = I_SH // NCORES       # 512

P = 128
KT = H // P                    # 16
ITE = I_EXP // P               # 8
ITS = I_SLICE // P             # 4
TB = 512
NBLK = T // TB                 # 16
HH = 512
NHH = H // HH                  # 4
NRANGE = 4
TRANGE = T // NRANGE           # 2048

F32 = mybir.dt.float32
F16 = mybir.dt.float16
I16 = mybir.dt.int16
ALU = mybir.AluOpType
ACTF = mybir.ActivationFunctionType


def build_kernel(pr, group_counts, num_devices=NCORES, with_rs=True,
                 do_expert=True, do_shared=True, debug_out=False):
    """pr: per-range padded counts (multiples of 128, same on all cores).
    group_counts: per-128-group valid-row count (0 => skip scatter)."""
    cap = int(np.ceil(sum(pr) / TB) * TB)          # block-padded capacity
    n_groups = cap // P
    nbe = cap // TB
    # group -> range mapping from pr prefix sums
    g_range = []
    acc = 0
    bounds = np.cumsum(pr)
    for g in range(n_groups):
        lo = g * P
        r = int(np.searchsorted(bounds, lo, side="right"))
        g_range.append(r if r < NRANGE else None)   # None => trailing pad group

    nc = bacc.Bacc(
        "TRN2", target_bir_lowering=False, debug=False, enable_asserts=False,
        num_devices=num_devices, num_swdge_queues=4,
    )
    xT16 = nc.dram_tensor("xT16", [KT, P, T], F16, kind="ExternalInput").ap()
    xTe16 = nc.dram_tensor("xTe16", [KT, P, cap], F16, kind="ExternalInput").ap()
    w9g = nc.dram_tensor("w9g", [KT, P, E], F16, kind="ExternalInput").ap()
    wseg = nc.dram_tensor("wseg", [KT, P, 1], F16, kind="ExternalInput").ap()
    w1e = nc.dram_tensor("w1e", [KT, P, I_EXP], F16, kind="ExternalInput").ap()
    w2e = nc.dram_tensor("w2e", [KT, P, I_EXP], F16, kind="ExternalInput").ap()
    w3e = nc.dram_tensor("w3e", [ITE, P, H], F16, kind="ExternalInput").ap()
    w1s = nc.dram_tensor("w1s", [KT, P, I_SLICE], F16, kind="ExternalInput").ap()
    w2s = nc.dram_tensor("w2s", [KT, P, I_SLICE], F16, kind="ExternalInput").ap()
    w3s = nc.dram_tensor("w3s", [ITS, P, H], F16, kind="ExternalInput").ap()
    sel8d = nc.dram_tensor("sel8", [E, 1], F32, kind="ExternalInput").ap()
    idx16d = nc.dram_tensor("idx16", [P, cap // 16], I16, kind="ExternalInput").ap()
    out_shard = nc.dram_tensor(
        "out_shard", [NRANGE, TRANGE // NCORES, H], F32, kind="ExternalOutput"
    ).ap()

    with tile.TileContext(nc) as tc, ExitStack() as ctx:
        dram = ctx.enter_context(tc.tile_pool(name="dram", bufs=1, space="DRAM"))
        # rows [TRANGE, TRANGE+P) are a garbage region for equalizing pads
        partials = [
            dram.tile([TRANGE + P, H], F32, tag=f"partial{r}", name=f"partial{r}")
            for r in range(NRANGE)
        ]

        # Zeroing 68MB up front would put ~190us of DMA ahead of the first
        # weight/activation loads. Instead emit each range's zero DMAs just
        # before the expert block whose scatters first need that range
        # (glist is range-ordered), so zeroing hides behind compute.
        sbZ = ctx.enter_context(tc.tile_pool(name="sbZ", bufs=1))
        ztile = sbZ.tile([P, H], F32, tag="ztile")
        nc.vector.memset(ztile[:], 0.0)
        zeroed = [False] * NRANGE

        def zero_range(r):
            if r is None or zeroed[r]:
                return
            zeroed[r] = True
            for i in range(TRANGE // P):
                nc.sync.dma_start(partials[r][i * P : (i + 1) * P, :], ztile[:])

        if do_expert:
            _expert_pass(nc, tc, partials, xTe16, w9g, w1e, w2e, w3e, sel8d,
                         idx16d, cap, nbe, g_range, group_counts, zero_range)
        for r in range(NRANGE):
            zero_range(r)
        if do_shared:
            _shared_pass(nc, tc, partials, xT16, wseg, w1s, w2s, w3s)

        # ---- per-range ReduceScatter + output -----------------------------
        with tc.tile_pool(name="dramR", bufs=1, space="DRAM") as dramR:
            if with_rs:
                for r in range(NRANGE):
                    rs_out = dramR.tile(
                        [TRANGE // NCORES, H], F32, tag=f"rsout{r}", name=f"rsout{r}"
                    )
                    nc.gpsimd.collective_compute(
                        "ReduceScatter",
                        ALU.add,
                        replica_groups=[list(range(NCORES))],
                        ins=[partials[r][0:TRANGE, :].opt()],
                        outs=[rs_out.opt()],
                    )
                    nc.sync.dma_start(out_shard[r], rs_out[:])
            elif debug_out:
                dbg = nc.dram_tensor(
                    "partial_dbg", [T, H], F32, kind="ExternalOutput"
                ).ap()
                for r in range(NRANGE):
                    nc.sync.dma_start(
                        dbg[r * TRANGE : (r + 1) * TRANGE, :],
                        partials[r][0:TRANGE, :],
                    )
            else:
                for r in range(NRANGE):
                    nc.sync.dma_start(
                        out_shard[r], partials[r][0 : TRANGE // NCORES, :]
                    )

    nc.compile()
    return nc


def _expert_pass(nc, tc, partials, xTe16, w9g, w1e, w2e, w3e, sel8d, idx16d,
                 cap, nbe, g_range, group_counts, zero_range):
    with (
        tc.tile_pool(name="cstE", bufs=1) as cst,
        tc.tile_pool(name="sbXE", bufs=2) as sbX,
        tc.tile_pool(name="sbHE", bufs=1) as sbH,
        tc.tile_pool(name="sbTE", bufs=2) as sbT,
        tc.tile_pool(name="sbYE", bufs=1) as sbY,
        tc.tile_pool(name="psE", bufs=2, space="PSUM") as ps,
    ):
        w1sb = cst.tile([P, KT, I_EXP], F16, tag="w1sb")
        nc.sync.dma_start(w1sb[:], w1e.rearrange("k p n -> p k n"))
        w2sb = cst.tile([P, KT, I_EXP], F16, tag="w2sb")
        nc.sync.dma_start(w2sb[:], w2e.rearrange("k p n -> p k n"))
        w9sb = cst.tile([P, KT, E], F16, tag="w9sb")
        nc.sync.dma_start(w9sb[:], w9g.rearrange("k p n -> p k n"))
        sel8sb = cst.tile([E, 1], F32, tag="sel8sb")
        nc.sync.dma_start(sel8sb[:], sel8d)
        idxsb = cst.tile([P, cap // 16], I16, tag="idxsb")
        nc.sync.dma_start(idxsb[:], idx16d)
        w3esb = cst.tile([P, ITE, H], F16, tag="w3esb")
        nc.sync.dma_start(w3esb[:], w3e.rearrange("i p h -> p i h"))

        for eb in range(nbe):
            # zero the ranges this block's scatters will touch
            for g in range(eb * (TB // P), (eb + 1) * (TB // P)):
                zero_range(g_range[g])
            bsl = slice(eb * TB, (eb + 1) * TB)
            xb = sbX.tile([P, KT, TB], F16, tag="xbe")
            nc.sync.dma_start(xb[:], xTe16[:, :, bsl].rearrange("k p t -> p k t"))

            # recompute this expert's routing weight for gathered tokens:
            # comb = softmax(logits)[e]  (token is in top-2 by construction)
            ps_l = ps.tile([E, TB], F32, tag="ps_l")
            for ko in range(KT):
                nc.tensor.matmul(
                    ps_l[:], w9sb[:, ko, :], xb[:, ko],
                    start=(ko == 0), stop=(ko == KT - 1),
                )
            ex = sbT.tile([E, TB], F32, tag="ex")
            nc.scalar.activation(ex[:], ps_l[:], ACTF.Exp)
            sm = sbT.tile([E, TB], F32, tag="sm")
            nc.gpsimd.partition_all_reduce(
                sm[:], ex[:], channels=E, reduce_op=bass_isa.ReduceOp.add
            )
            rc = sbT.tile([E, TB], F32, tag="rc")
            nc.vector.reciprocal(rc[:], sm[:])
            pr_ = sbT.tile([E, TB], F32, tag="pr")
            nc.vector.tensor_mul(out=pr_[:], in0=ex[:], in1=rc[:])
            nc.vector.tensor_tensor(
                pr_[:], pr_[:], sel8sb[:, 0:1].to_broadcast([E, TB]), ALU.mult
            )
            ce = sbT.tile([E, TB], F32, tag="ce")
            nc.gpsimd.partition_all_reduce(
                ce[:], pr_[:], channels=E, reduce_op=bass_isa.ReduceOp.add
            )
            bc_e = sbT.tile([P, TB], F32, tag="bc_e")
            nc.gpsimd.partition_broadcast(bc_e[:], ce[0:1, :], channels=P)

            hc = sbH.tile([P, ITE, TB], F16, tag="hce")
            for it in range(ITE):
                isl = slice(it * P, (it + 1) * P)
                psG = ps.tile([P, TB], F32, tag="psG")
                psU = ps.tile([P, TB], F32, tag="psU")
                for ko in range(KT):
                    nc.tensor.matmul(
                        psG[:], w1sb[:, ko, isl], xb[:, ko],
                        start=(ko == 0), stop=(ko == KT - 1),
                    )
                for ko in range(KT):
                    nc.tensor.matmul(
                        psU[:], w2sb[:, ko, isl], xb[:, ko],
                        start=(ko == 0), stop=(ko == KT - 1),
                    )
                sg = sbT.tile([P, TB], F32, tag="sg")
                nc.scalar.activation(sg[:], psG[:], ACTF.Silu)
                nc.vector.tensor_mul(out=sg[:], in0=sg[:], in1=psU[:])
                nc.vector.tensor_tensor(hc[:, it], sg[:], bc_e[:], ALU.mult)

            # token-major down-proj: yf[tok, h]
            yf = sbY.tile([P, TB // P, NHH * HH], F32, tag="yf")
            for hh in range(NHH):
                for ts_ in range(TB // P):
                    psY = ps.tile([P, HH], F32, tag="psY")
                    for it in range(ITE):
                        nc.tensor.matmul(
                            psY[:],
                            hc[:, it, ts_ * P : (ts_ + 1) * P],
                            w3esb[:, it, hh * HH : (hh + 1) * HH],
                            start=(it == 0), stop=(it == ITE - 1),
                        )
                    nc.vector.tensor_copy(
                        yf[:, ts_, hh * HH : (hh + 1) * HH], psY[:]
                    )
            # scatter-add 128-row groups into their range's partial
            for ts_ in range(TB // P):
                g = eb * (TB // P) + ts_
                r = g_range[g]
                nvalid = group_counts[g]
                if r is None or nvalid == 0:
                    continue
                nc.gpsimd.dma_scatter_add(
                    out_ap=partials[r][:, :],
                    in_ap=yf[:, ts_ : ts_ + 1, :],
                    idxs_ap=idxsb[:, g * 8 : (g + 1) * 8],
                    num_idxs=P,
                    num_idxs_reg=int(nvalid),
                    elem_size=H,
                    queue_num=g % 4,
                )


def _shared_pass(nc, tc, partials, xT16, wseg, w1s, w2s, w3s):
    with (
        tc.tile_pool(name="cstS", bufs=1) as cst,
        tc.tile_pool(name="sbXS", bufs=2) as sbX,
        tc.tile_pool(name="sbHS", bufs=2) as sbH,
        tc.tile_pool(name="sbTS", bufs=3) as sbT,
        tc.tile_pool(name="sbYS", bufs=2) as sbY,
        tc.tile_pool(name="psS", bufs=2, space="PSUM") as ps,
    ):
        w1sb = cst.tile([P, KT, I_SLICE], F16, tag="w1sbs")
        nc.sync.dma_start(w1sb[:], w1s.rearrange("k p n -> p k n"))
        w2sb = cst.tile([P, KT, I_SLICE], F16, tag="w2sbs")
        nc.sync.dma_start(w2sb[:], w2s.rearrange("k p n -> p k n"))
        wgsb = cst.tile([P, KT, 1], F16, tag="wgsb")
        nc.sync.dma_start(wgsb[:], wseg.rearrange("k p n -> p k n"))
        # shared-expert down weights are small enough to keep resident
        w3sb = cst.tile([P, ITS, H], F16, tag="w3sb")
        nc.sync.dma_start(w3sb[:], w3s.rearrange("i p h -> p i h"))

        for b in range(NBLK):
            bsl = slice(b * TB, (b + 1) * TB)
            rng_i = (b * TB) // TRANGE
            xb = sbX.tile([P, KT, TB], F16, tag="xbs")
            nc.sync.dma_start(xb[:], xT16[:, :, bsl].rearrange("k p t -> p k t"))

            ps_g = ps.tile([1, TB], F32, tag="ps_g")
            for ko in range(KT):
                nc.tensor.matmul(
                    ps_g[:], wgsb[:, ko, :], xb[:, ko],
                    start=(ko == 0), stop=(ko == KT - 1),
                )
            srow = sbT.tile([1, TB], F32, tag="srow")
            nc.scalar.activation(srow[:], ps_g[:], ACTF.Sigmoid)
            bc_s = sbT.tile([P, TB], F32, tag="bc_s")
            nc.gpsimd.partition_broadcast(bc_s[:], srow[:], channels=P)

            hc = sbH.tile([P, ITS, TB], F16, tag="hcs")
            for it in range(ITS):
                isl = slice(it * P, (it + 1) * P)
                psG = ps.tile([P, TB], F32, tag="psGs")
                psU = ps.tile([P, TB], F32, tag="psUs")
                for ko in range(KT):
                    nc.tensor.matmul(
                        psG[:], w1sb[:, ko, isl], xb[:, ko],
                        start=(ko == 0), stop=(ko == KT - 1),
                    )
                for ko in range(KT):
                    nc.tensor.matmul(
                        psU[:], w2sb[:, ko, isl], xb[:, ko],
                        start=(ko == 0), stop=(ko == KT - 1),
                    )
                sg = sbT.tile([P, TB], F32, tag="sgs")
                nc.scalar.activation(sg[:], psG[:], ACTF.Silu)
                nc.vector.tensor_mul(out=sg[:], in0=sg[:], in1=psU[:])
                nc.vector.tensor_tensor(hc[:, it], sg[:], bc_s[:], ALU.mult)

            # stage the whole block's [TB, H] in SBUF, then one accum-DMA
            # (per-tile accum-DMAs each hold the GPSIMD engine ~1us for
            # SWDGE descriptor generation -- 256 of them serialize)
            yb = sbY.tile([P, TB // P, H], F32, tag="yblk")
            for hh in range(NHH):
                for ts_ in range(TB // P):
                    psY = ps.tile([P, HH], F32, tag="psYs")
                    for it in range(ITS):
                        nc.tensor.matmul(
                            psY[:],
                            hc[:, it, ts_ * P : (ts_ + 1) * P],
                            w3sb[:, it, hh * HH : (hh + 1) * HH],
                            start=(it == 0), stop=(it == ITS - 1),
                        )
                    nc.vector.tensor_copy(
                        yb[:, ts_, hh * HH : (hh + 1) * HH], psY[:]
                    )
            row0 = (b * TB) % TRANGE
            nc.gpsimd.dma_start(
                partials[rng_i][row0 : row0 + TB, :].rearrange(
                    "(a p) h -> p a h", p=P
                ),
                yb[:],
                accum_op=ALU.add,
            )


def route_host(inputs):
    """Routing decision only (indices); all values are recomputed on device."""
    x = np.asarray(inputs["hidden_states"], np.float64).reshape(T, H)
    gw = np.asarray(inputs["gate_w"], np.float64)
    logits = x @ gw.T
    p = np.exp(logits - logits.max(-1, keepdims=True))
    p /= p.sum(-1, keepdims=True)
    order = np.argsort(-p, axis=-1, kind="stable")
    top2 = order[:, :TOP_K]
    per_core = []
    for e in range(NCORES):
        toks = np.where((top2 == e).any(-1))[0]
        per_range = [
            toks[(toks >= r * TRANGE) & (toks < (r + 1) * TRANGE)] - r * TRANGE
            for r in range(NRANGE)
        ]
        per_core.append(per_range)
    # per-range padded counts: max over cores, rounded up to 128
    pr = [
        int(np.ceil(max(len(per_core[c][r]) for c in range(NCORES)) / P) * P)
        for r in range(NRANGE)
    ]
    return per_core, pr


def _derive_params(inputs):
    """Per-group scatter counts must be identical across cores (one SPMD
    program).  Each group's count = max over cores; cores with fewer tokens
    pad the window with index TRANGE (the garbage row region), so every core
    has exactly `count` non-negative entries followed by -1s."""
    per_core, pr = route_host(inputs)
    cap = int(np.ceil(sum(pr) / TB) * TB)
    n_groups = cap // P
    bounds = np.cumsum(pr)
    group_counts = []
    for g in range(n_groups):
        lo = g * P
        r = int(np.searchsorted(bounds, lo, side="right"))
        if r >= NRANGE:
            group_counts.append(0)
            continue
        seg_lo = lo - (int(bounds[r - 1]) if r > 0 else 0)
        mx = max(
            min(max(len(per_core[c][r]) - seg_lo, 0), P) for c in range(NCORES)
        )
        group_counts.append(int(mx))
    return per_core, pr, cap, group_counts


def build_idx_and_gather(per_range_tok, pr, cap, group_counts):
    """Per-core: padded index list (range-relative; surplus entries within a
    group's count point at the garbage row TRANGE; -1 beyond) + absolute
    gather columns (pads gather token 0)."""
    glist = np.full((cap,), -1, np.int16)
    gcols = np.zeros((cap,), np.int64)
    bounds = np.cumsum(pr)
    off = 0
    for r in range(NRANGE):
        toks = per_range_tok[r]
        glist[off : off + len(toks)] = toks.astype(np.int16)
        gcols[off : off + len(toks)] = toks + r * TRANGE
        off += pr[r]
    # within each group, raise entries [own_count, group_count) to garbage row
    for g in range(cap // P):
        cnt = group_counts[g]
        w = glist[g * P : (g + 1) * P]
        pad = (w < 0) & (np.arange(P) < cnt)
        w[pad] = TRANGE
    idx16 = np.empty((P, cap // 16), np.int16)
    wrapped = glist.reshape(cap // 16, 16).T        # [16, cap/16]
    for k in range(8):
        idx16[k * 16 : (k + 1) * 16] = wrapped
    return glist, gcols, idx16


def make_in_maps(inputs):
    hs = np.ascontiguousarray(inputs["hidden_states"], dtype=np.float32)
    x = hs.reshape(T, H)
    xT16_flat = np.ascontiguousarray(x.T).astype(np.float16)
    xT16 = xT16_flat.reshape(KT, P, T)

    gate_w = np.asarray(inputs["gate_w"], np.float32)
    seg_w = np.asarray(inputs["shared_expert_gate_w"], np.float32)
    w9g = np.ascontiguousarray(gate_w.T).astype(np.float16).reshape(KT, P, E)
    wseg = np.ascontiguousarray(seg_w.T).astype(np.float16).reshape(KT, P, 1)

    egw = np.asarray(inputs["expert_gate_w"], np.float32)
    euw = np.asarray(inputs["expert_up_w"], np.float32)
    edw = np.asarray(inputs["expert_down_w"], np.float32)
    sgw = np.asarray(inputs["shared_gate_w"], np.float32)
    suw = np.asarray(inputs["shared_up_w"], np.float32)
    sdw = np.asarray(inputs["shared_down_w"], np.float32)

    per_core, pr, cap, group_counts = _derive_params(inputs)

    in_maps = []
    for c in range(NCORES):
        ssl = slice(c * I_SLICE, (c + 1) * I_SLICE)
        glist, gcols, idx16 = build_idx_and_gather(
            per_core[c], pr, cap, group_counts
        )
        xTe16 = np.ascontiguousarray(xT16_flat[:, gcols]).reshape(KT, P, cap)
        sel8 = np.zeros((E, 1), np.float32)
        sel8[c, 0] = 1.0
        in_maps.append(
            {
                "xT16": xT16,
                "xTe16": xTe16,
                "w9g": w9g,
                "wseg": wseg,
                "w1e": np.ascontiguousarray(egw[c].T).astype(np.float16).reshape(KT, P, I_EXP),
                "w2e": np.ascontiguousarray(euw[c].T).astype(np.float16).reshape(KT, P, I_EXP),
                "w3e": np.ascontiguousarray(edw[c].T).astype(np.float16).reshape(ITE, P, H),
                "w1s": np.ascontiguousarray(sgw[ssl].T).astype(np.float16).reshape(KT, P, I_SLICE),
                "w2s": np.ascontiguousarray(suw[ssl].T).astype(np.float16).reshape(KT, P, I_SLICE),
                "w3s": np.ascontiguousarray(sdw[:, ssl].T).astype(np.float16).reshape(ITS, P, H),
                "sel8": sel8,
                "idx16": idx16,
            }
        )
    return in_maps, pr, group_counts


def assemble_output(results):
    out = np.empty((T, H), np.float32)
    rows = TRANGE // NCORES
    for c in range(NCORES):
        sh = results[c]["out_shard"]
        for r in range(NRANGE):
            base = r * TRANGE + c * rows
            out[base : base + rows] = sh[r]
    return out.reshape(B, S, H)


_nc_cache = {}


def kernel(**inputs) -> np.ndarray:
    in_maps, pr, group_counts = make_in_maps(inputs)
    key = (tuple(pr), tuple(group_counts))
    if key not in _nc_cache:
        nc = build_kernel(pr, group_counts)
        nc.m = get_hw_module(nc.m)
        _nc_cache[key] = nc
    nc = _nc_cache[key]
    res = bass_utils.run_bass_kernel_spmd(
        nc, in_maps, core_ids=list(range(NCORES))
    )
    return assemble_output(res.results)
